# revision 21
# baseline (speedup 1.0000x reference)
"""NNUE-style embedding-lookup + tiny-MLP kernel for Trainium2 (8 NeuronCores).

Data-parallel over the batch dim: each of the 8 cores handles 2048 of the
16384 batch positions.

Key optimizations:
- The first MLP layer is linear, so the 1 KB embedding rows are folded with
  w1 on the host: proj = emb @ w1.T [49152, 32] in bf16, padded to 128 bf16
  columns (256 B rows -- the dma_gather minimum element size). The device
  gathers 256 B rows instead of 1 KB and the w1 matmul disappears.
- SWDGE descriptor generation (~7.6 ns/index/queue-pair, 4 queues, ~1.4 us
  fixed per call) is the throughput wall; payload bytes are nearly free.
  Descriptors trigger at the END of each call's generation and calls
  dispatch in in-order windows of 4, so the schedule uses a small solo
  leader, a clean 4-call ramp window, full-size middles, and a small last
  window.
- Feature accumulation runs on DVE (strided tensor_reduce over the gathered
  slots + adds), not on PE identity matmuls: the consumer chain per t-block
  is ~1.5 us (2-3 DVE reduces, 1 PE transpose, 1 fused DVE bias+relu into a
  persistent h1 buffer), so the last t-blocks' chains don't pile up after
  the final gather call.
- The 32->32->1 MLP runs in bf16, batched over 512-batch chunks, emitted
  mid-run as soon as each chunk's 4 t-blocks are done.
"""

import numpy as np

INPUT_DIM = 49152
PROJ_W = 128         # padded bf16 proj row: 32 real + 96 pad = 256 B
BATCH = 16384
F = 32               # active features per position
N_CORES = 8
B_CORE = BATCH // N_CORES          # 2048 batch positions per core
BIAS = 16384                       # index bias for int16 gather
PAD = 16                           # per-call idx pad (keeps last index >= 0)
NT = B_CORE // 128                 # 16 t-blocks (128 batches each)
N_SLOTS = NT * F                   # 512 gather slots of 128 lookups

# Call sizes in slots (1 slot = 128 lookups): small solo leader, then
# 8-slot calls throughout. 1040-index calls generate at ~3.5 ns/idx on the
# Q7 pairs vs ~5.7+ ns/idx for 2064-index calls (descriptor-ring capacity
# cliff), so many small calls beat few large ones.
SLOT_SIZES = [4] + [8] * 63 + [4]
assert sum(SLOT_SIZES) == N_SLOTS


def _call_table():
    """Flat call list: (col, cw, n_idx, n_slots_real, global_slot0)."""
    calls = []
    col = 0
    gs = 0
    for ns in SLOT_SIZES:
        n_idx = ns * 128 + PAD
        cw = n_idx // 16
        stride = ((cw + 31) // 32) * 32  # 64 B-aligned call starts
        calls.append((col, cw, n_idx, ns, gs))
        col += stride
        gs += ns
    return calls, col


CALLS, IDX_COLS = _call_table()


def _ranges():
    """Per t-block: list of (call_idx, local_slot0, length) covering its
    32 slots, split only at call boundaries."""
    out = [[] for _ in range(NT)]
    for ci, (col, cw, n_idx, ns, gs0) in enumerate(CALLS):
        s = 0
        while s < ns:
            t = (gs0 + s) // F
            ln = min(ns - s, F - ((gs0 + s) % F))
            out[t].append((ci, s, ln))
            s += ln
    return out


BLOCK_RANGES = _ranges()

_nc_cache = None


def _build():
    import os
    import concourse.bacc as bacc
    import concourse.mybir as mybir
    import concourse.tile as tile

    stage = os.environ.get("KERNEL_STAGE", "full")

    f32 = mybir.dt.float32
    bf16 = mybir.dt.bfloat16
    i16 = mybir.dt.int16
    AF = mybir.ActivationFunctionType
    ADD = mybir.AluOpType.add
    MAX = mybir.AluOpType.max

    nc = bacc.Bacc(None, target_bir_lowering=False, debug=False, num_swdge_queues=4)
    emb = nc.dram_tensor("emb", [INPUT_DIM, PROJ_W], bf16, kind="ExternalInput")
    idx = nc.dram_tensor("idx", [128, IDX_COLS], i16, kind="ExternalInput")
    ident = nc.dram_tensor("ident", [128, 128], f32, kind="ExternalInput")
    b1 = nc.dram_tensor("b1", [32, 1], f32, kind="ExternalInput")
    w2l = nc.dram_tensor("w2l", [32, 32], bf16, kind="ExternalInput")
    b2 = nc.dram_tensor("b2", [32, 1], f32, kind="ExternalInput")
    w3l = nc.dram_tensor("w3l", [32, 1], bf16, kind="ExternalInput")
    b3 = nc.dram_tensor("b3", [1, 1], f32, kind="ExternalInput")
    out = nc.dram_tensor("out", [1, B_CORE], f32, kind="ExternalOutput")

    S_MAX = max(SLOT_SIZES) + 1  # slots in the largest gather tile (+pad)

    with tile.TileContext(nc) as tc:
        with (
            tc.tile_pool(name="const", bufs=1) as cpool,
            tc.tile_pool(name="g", bufs=16) as gpool,
            tc.tile_pool(name="xs", bufs=3) as xspool,
            tc.tile_pool(name="hs", bufs=4) as hspool,
            tc.tile_pool(name="xtp", bufs=2, space="PSUM") as xtppool,
            tc.tile_pool(name="mp", bufs=4, space="PSUM") as mppool,
        ):
            # idx slices load first; the leader call only needs the first
            # (small) slice, so it lands early (per-range dep tracking).
            idx_t = cpool.tile([128, IDX_COLS], i16)
            first = ((CALLS[0][1] + 31) // 32) * 32
            bounds = [0, first]
            rest = IDX_COLS - first
            for k in range(7):
                bounds.append(first + ((rest * (k + 1) // 7 + 31) // 32) * 32)
            bounds[-1] = IDX_COLS
            for lo, hi in zip(bounds[:-1], bounds[1:]):
                if lo < hi:
                    nc.sync.dma_start(idx_t[:, lo:hi], idx[:, lo:hi])
            id_t = cpool.tile([128, 128], f32)
            nc.sync.dma_start(id_t[:], ident[:])
            b1_t = cpool.tile([32, 1], f32)
            nc.sync.dma_start(b1_t[:], b1[:])
            w2l_t = cpool.tile([32, 32], bf16)
            nc.sync.dma_start(w2l_t[:], w2l[:])
            b2_t = cpool.tile([32, 1], f32)
            nc.sync.dma_start(b2_t[:], b2[:])
            w3l_t = cpool.tile([32, 1], bf16)
            nc.sync.dma_start(w3l_t[:], w3l[:])
            b3_t = cpool.tile([1, 1], f32)
            nc.sync.dma_start(b3_t[:], b3[:])
            out_t = cpool.tile([1, B_CORE], f32)

            # Persistent relu(x@w1.T+b1) buffer (bf16): per-t-block chains
            # end here; the 32->32->1 MLP runs batched per 4 t-blocks.
            h1all = cpool.tile([32, B_CORE], bf16)

            # Pre-load the Tanh activation table (~1.3 us) so it doesn't
            # land in the critical tail at the first real Tanh.
            warm_t = hspool.tile([1, 1], f32, tag="hs")
            nc.scalar.activation(warm_t[:], b3_t[:], AF.Tanh)

            # Shared num_idxs registers: one MOVE each instead of one per
            # call (each MOVE costs ~400 ns of GpSimd sequencer time).
            nreg = {}
            for sz in sorted({c[2] for c in CALLS}):
                nreg[sz] = nc.gpsimd.to_reg(sz)

            g_tiles = [None] * len(CALLS)

            def mlp_tail(k):
                """Batched bf16 32->32->1 MLP over 512 batches (4 t-blocks).

                bf16 matmuls run 1 cycle/row; DVE does the fused bias+relu;
                only the Tanh needs ScalarE.
                """
                sl = slice(512 * k, 512 * (k + 1))
                h2p = mppool.tile([32, 512], f32, tag="mp")
                nc.tensor.matmul(
                    h2p[:], lhsT=w2l_t[:], rhs=h1all[:, sl], start=True, stop=True
                )
                h2s = hspool.tile([32, 512], bf16, tag="h2s")
                nc.vector.tensor_scalar(
                    out=h2s[:], in0=h2p[:], scalar1=b2_t[:], scalar2=0.0,
                    op0=ADD, op1=MAX,
                )
                yp = mppool.tile([1, 512], f32, tag="mp")
                nc.tensor.matmul(
                    yp[:], lhsT=w3l_t[:], rhs=h2s[:], start=True, stop=True
                )
                nc.scalar.activation(out_t[:, sl], yp[:], AF.Tanh, bias=b3_t[:])

            def consume_block(t):
                """Per-t-block consumer: DVE reduces over the gathered slots
                (first 32 columns of each 256 B row), a PE transpose, and a
                fused DVE bias+relu into h1all."""
                x_sb = None
                for ci, s0, ln in BLOCK_RANGES[t]:
                    g = g_tiles[ci]
                    xr = xspool.tile([128, 32], f32, tag="xs")
                    nc.vector.tensor_reduce(
                        out=xr[:],
                        in_=g[:, s0 : s0 + ln, 0:32].rearrange("p s e -> p e s"),
                        axis=mybir.AxisListType.X,
                        op=ADD,
                    )
                    if x_sb is None:
                        x_sb = xr
                    else:
                        nc.vector.tensor_tensor(
                            out=x_sb[:], in0=x_sb[:], in1=xr[:], op=ADD
                        )
                xt_p = xtppool.tile([32, 128], f32, tag="xtp")
                nc.tensor.transpose(xt_p[:], x_sb[:], id_t[:])
                nc.vector.tensor_scalar(
                    out=h1all[:, 128 * t : 128 * (t + 1)],
                    in0=xt_p[:],
                    scalar1=b1_t[:],
                    scalar2=0.0,
                    op0=ADD,
                    op1=MAX,
                )
                if t % 4 == 3:
                    mlp_tail(t // 4)

            # Gather calls; consume each t-block (one block late, software
            # pipelined) once the call containing its last slot is issued.
            next_t = 0
            for ci, (col, cw, n_idx, ns, gs0) in enumerate(CALLS):
                n_slots = (n_idx + 127) // 128
                g = gpool.tile([128, S_MAX, PROJ_W], bf16, tag="g")
                g_tiles[ci] = g
                nc.gpsimd.dma_gather(
                    g[:, :n_slots, :],
                    emb[BIAS:, :],
                    idx_t[:, col : col + cw],
                    n_idx,
                    nreg[n_idx],
                    PROJ_W,
                    single_packet=False,
                    queue_num=ci % 4,
                )
                if stage == "gather":
                    nc.sync.dma_start(
                        out[:, (ci % 16) * 128 : (ci % 16) * 128 + 128],
                        g[0:1, 0:2, :].bitcast(f32).rearrange("a b c -> a (b c)"),
                    )
                    continue
                # blocks fully covered by calls 0..ci, minus one for swp
                done_t = (gs0 + ns) // F
                while next_t < done_t - 1:
                    consume_block(next_t)
                    next_t += 1
            if stage == "full":
                while next_t < NT:
                    consume_block(next_t)
                    next_t += 1
                nc.sync.dma_start(out[:], out_t[:])
    nc.compile()
    return nc


def _get_nc():
    global _nc_cache
    if _nc_cache is None:
        _nc_cache = _build()
    return _nc_cache


def _prep_indices(shard: np.ndarray) -> np.ndarray:
    """[F, B_CORE] int -> [128, IDX_COLS] int16 device layout.

    Global slot s = t*32 + f holds feature f for the 128 batches of t-block
    t; the feature-sum is a DVE reduction over slots. Indices are biased by
    -BIAS, padded with PAD zeros per call (row BIAS stays non-negative so
    the Q7 truncation loop is a no-op), laid out [16, CW] wrapped,
    replicated across the 8 Q7 core groups, and 64 B-aligned per call.
    """
    arr = shard.reshape(F, NT, 128)  # [f, t, b_in]
    stream = arr.transpose(1, 0, 2).astype(np.int64) - BIAS  # [t, f, b_in]
    stream = stream.reshape(N_SLOTS, 128)  # global slot-major
    outa = np.zeros((128, IDX_COLS), np.int16)
    for col, cw, n_idx, ns, gs0 in CALLS:
        lst = np.zeros(n_idx, np.int64)
        lst[: ns * 128] = stream[gs0 : gs0 + ns].reshape(-1)
        lay = lst.reshape(cw, 16).T  # [16, cw]
        outa[:, col : col + cw] = np.tile(lay, (8, 1))
    return outa


def _in_maps(inputs):
    import ml_dtypes

    indices = np.asarray(inputs["indices"])
    emb = np.asarray(inputs["emb"], dtype=np.float32)
    w1 = np.asarray(inputs["w1"], dtype=np.float32)
    b1 = np.asarray(inputs["b1"], dtype=np.float32)
    w2 = np.asarray(inputs["w2"], dtype=np.float32)
    b2 = np.asarray(inputs["b2"], dtype=np.float32)
    w3 = np.asarray(inputs["w3"], dtype=np.float32)
    b3 = np.asarray(inputs["b3"], dtype=np.float32)

    # Fold the first (linear) MLP layer into the table: 256 B bf16 rows.
    projp = np.zeros((INPUT_DIM, PROJ_W), ml_dtypes.bfloat16)
    projp[:, :32] = (emb @ w1.T).astype(ml_dtypes.bfloat16)

    common = {
        "emb": projp,
        "ident": np.eye(128, dtype=np.float32),
        "b1": b1.reshape(32, 1),
        "w2l": np.ascontiguousarray(w2.T).astype(ml_dtypes.bfloat16),
        "b2": b2.reshape(32, 1),
        "w3l": np.ascontiguousarray(w3.T).astype(ml_dtypes.bfloat16),
        "b3": b3.reshape(1, 1),
    }
    in_maps = []
    for c in range(N_CORES):
        shard = indices[:, c * B_CORE : (c + 1) * B_CORE]
        in_maps.append({**common, "idx": _prep_indices(shard)})
    return in_maps


def kernel(**inputs) -> np.ndarray:
    from concourse.bass_utils import run_bass_kernel_spmd

    nc = _get_nc()
    res = run_bass_kernel_spmd(nc, _in_maps(inputs), core_ids=list(range(N_CORES)))
    ys = [np.asarray(res.results[c]["out"]).reshape(B_CORE) for c in range(N_CORES)]
    return np.concatenate(ys).reshape(BATCH, 1).astype(np.float32)


# revision 23
# speedup vs baseline: 1.1561x; 1.1561x over previous
"""NNUE-style embedding-lookup + tiny-MLP kernel for Trainium2 (8 NeuronCores).

Data-parallel over the batch dim: each of the 8 cores handles 2048 of the
16384 batch positions.

Key optimizations:
- The first MLP layer is linear, so the 1 KB embedding rows are folded with
  w1 on the host: proj = emb @ w1.T [49152, 32] in bf16, padded to 128 bf16
  columns (256 B rows -- the dma_gather minimum element size). The device
  gathers 256 B rows instead of 1 KB and the w1 matmul disappears.
- SWDGE descriptor generation (~7.6 ns/index/queue-pair, 4 queues, ~1.4 us
  fixed per call) is the throughput wall; payload bytes are nearly free.
  Descriptors trigger at the END of each call's generation and calls
  dispatch in in-order windows of 4, so the schedule uses a small solo
  leader, a clean 4-call ramp window, full-size middles, and a small last
  window.
- Feature accumulation runs on DVE (strided tensor_reduce over the gathered
  slots + adds), not on PE identity matmuls: the consumer chain per t-block
  is ~1.5 us (2-3 DVE reduces, 1 PE transpose, 1 fused DVE bias+relu into a
  persistent h1 buffer), so the last t-blocks' chains don't pile up after
  the final gather call.
- The 32->32->1 MLP runs in bf16, batched over 512-batch chunks, emitted
  mid-run as soon as each chunk's 4 t-blocks are done.
"""

import numpy as np

INPUT_DIM = 49152
PROJ_W = 128         # padded bf16 proj row: 32 real + 96 pad = 256 B
BATCH = 16384
F = 32               # active features per position
N_CORES = 8
B_CORE = BATCH // N_CORES          # 2048 batch positions per core
BIAS = 16384                       # index bias for int16 gather
PAD = 16                           # per-call idx pad (keeps last index >= 0)
NT = B_CORE // 128                 # 16 t-blocks (128 batches each)
N_SLOTS = NT * F                   # 512 gather slots of 128 lookups

# Call sizes in slots (1 slot = 128 lookups). Measured dispatch behavior:
# calls dispatch in in-order windows of 4 (one per SWDGE queue), and each
# window costs gen(n_idx) + ~6 us of serialization, with gen ~6.8 ns/idx.
# Fewer, larger windows amortize the 6 us; the last window stays small so
# its data (descriptors only trigger at gen END) lands early for the tail.
# A tiny solo leader absorbs the first call's solo-dispatch penalty.
SLOT_SIZES = [2] + [44] * 8 + [32] * 4 + [8, 8, 8, 6]
assert sum(SLOT_SIZES) == N_SLOTS


def _call_table():
    """Flat call list: (col, cw, n_idx, n_slots_real, global_slot0)."""
    calls = []
    col = 0
    gs = 0
    for ns in SLOT_SIZES:
        n_idx = ns * 128 + PAD
        cw = n_idx // 16
        stride = ((cw + 31) // 32) * 32  # 64 B-aligned call starts
        calls.append((col, cw, n_idx, ns, gs))
        col += stride
        gs += ns
    return calls, col


CALLS, IDX_COLS = _call_table()


def _ranges():
    """Per t-block: list of (call_idx, local_slot0, length) covering its
    32 slots, split only at call boundaries."""
    out = [[] for _ in range(NT)]
    for ci, (col, cw, n_idx, ns, gs0) in enumerate(CALLS):
        s = 0
        while s < ns:
            t = (gs0 + s) // F
            ln = min(ns - s, F - ((gs0 + s) % F))
            out[t].append((ci, s, ln))
            s += ln
    return out


BLOCK_RANGES = _ranges()

_nc_cache = None


def _build():
    import os
    import concourse.bacc as bacc
    import concourse.mybir as mybir
    import concourse.tile as tile

    stage = os.environ.get("KERNEL_STAGE", "full")

    f32 = mybir.dt.float32
    bf16 = mybir.dt.bfloat16
    i16 = mybir.dt.int16
    AF = mybir.ActivationFunctionType
    ADD = mybir.AluOpType.add
    MAX = mybir.AluOpType.max

    nc = bacc.Bacc(None, target_bir_lowering=False, debug=False, num_swdge_queues=4)
    emb = nc.dram_tensor("emb", [INPUT_DIM, PROJ_W], bf16, kind="ExternalInput")
    idx = nc.dram_tensor("idx", [128, IDX_COLS], i16, kind="ExternalInput")
    ident = nc.dram_tensor("ident", [128, 128], f32, kind="ExternalInput")
    b1 = nc.dram_tensor("b1", [32, 1], f32, kind="ExternalInput")
    w2l = nc.dram_tensor("w2l", [32, 32], bf16, kind="ExternalInput")
    b2 = nc.dram_tensor("b2", [32, 1], f32, kind="ExternalInput")
    w3l = nc.dram_tensor("w3l", [32, 1], bf16, kind="ExternalInput")
    b3 = nc.dram_tensor("b3", [1, 1], f32, kind="ExternalInput")
    out = nc.dram_tensor("out", [1, B_CORE], f32, kind="ExternalOutput")

    S_MAX = max(SLOT_SIZES) + 1  # slots in the largest gather tile (+pad)

    with tile.TileContext(nc) as tc:
        with (
            tc.tile_pool(name="const", bufs=1) as cpool,
            tc.tile_pool(name="g", bufs=6) as gpool,
            tc.tile_pool(name="xs", bufs=3) as xspool,
            tc.tile_pool(name="hs", bufs=4) as hspool,
            tc.tile_pool(name="xtp", bufs=2, space="PSUM") as xtppool,
            tc.tile_pool(name="mp", bufs=4, space="PSUM") as mppool,
        ):
            # idx slices load first; the leader call only needs the first
            # (small) slice, so it lands early (per-range dep tracking).
            idx_t = cpool.tile([128, IDX_COLS], i16)
            first = ((CALLS[0][1] + 31) // 32) * 32
            bounds = [0, first]
            rest = IDX_COLS - first
            for k in range(7):
                bounds.append(first + ((rest * (k + 1) // 7 + 31) // 32) * 32)
            bounds[-1] = IDX_COLS
            for lo, hi in zip(bounds[:-1], bounds[1:]):
                if lo < hi:
                    nc.sync.dma_start(idx_t[:, lo:hi], idx[:, lo:hi])
            id_t = cpool.tile([128, 128], f32)
            nc.sync.dma_start(id_t[:], ident[:])
            b1_t = cpool.tile([32, 1], f32)
            nc.sync.dma_start(b1_t[:], b1[:])
            w2l_t = cpool.tile([32, 32], bf16)
            nc.sync.dma_start(w2l_t[:], w2l[:])
            b2_t = cpool.tile([32, 1], f32)
            nc.sync.dma_start(b2_t[:], b2[:])
            w3l_t = cpool.tile([32, 1], bf16)
            nc.sync.dma_start(w3l_t[:], w3l[:])
            b3_t = cpool.tile([1, 1], f32)
            nc.sync.dma_start(b3_t[:], b3[:])
            out_t = cpool.tile([1, B_CORE], f32)

            # Persistent relu(x@w1.T+b1) buffer (bf16): per-t-block chains
            # end here; the 32->32->1 MLP runs batched per 4 t-blocks.
            h1all = cpool.tile([32, B_CORE], bf16)

            # Pre-load the Tanh activation table (~1.3 us) so it doesn't
            # land in the critical tail at the first real Tanh.
            warm_t = hspool.tile([1, 1], f32, tag="hs")
            nc.scalar.activation(warm_t[:], b3_t[:], AF.Tanh)

            # Shared num_idxs registers: one MOVE each instead of one per
            # call (each MOVE costs ~400 ns of GpSimd sequencer time).
            nreg = {}
            for sz in sorted({c[2] for c in CALLS}):
                nreg[sz] = nc.gpsimd.to_reg(sz)

            g_tiles = [None] * len(CALLS)

            def mlp_tail(k):
                """Batched bf16 32->32->1 MLP over 512 batches (4 t-blocks).

                bf16 matmuls run 1 cycle/row; DVE does the fused bias+relu;
                only the Tanh needs ScalarE.
                """
                sl = slice(512 * k, 512 * (k + 1))
                h2p = mppool.tile([32, 512], f32, tag="mp")
                nc.tensor.matmul(
                    h2p[:], lhsT=w2l_t[:], rhs=h1all[:, sl], start=True, stop=True
                )
                h2s = hspool.tile([32, 512], bf16, tag="h2s")
                nc.vector.tensor_scalar(
                    out=h2s[:], in0=h2p[:], scalar1=b2_t[:], scalar2=0.0,
                    op0=ADD, op1=MAX,
                )
                yp = mppool.tile([1, 512], f32, tag="mp")
                nc.tensor.matmul(
                    yp[:], lhsT=w3l_t[:], rhs=h2s[:], start=True, stop=True
                )
                nc.scalar.activation(out_t[:, sl], yp[:], AF.Tanh, bias=b3_t[:])

            def consume_block(t):
                """Per-t-block consumer: DVE reduces over the gathered slots
                (first 32 columns of each 256 B row), a PE transpose, and a
                fused DVE bias+relu into h1all."""
                x_sb = None
                for ci, s0, ln in BLOCK_RANGES[t]:
                    g = g_tiles[ci]
                    xr = xspool.tile([128, 32], f32, tag="xs")
                    nc.vector.tensor_reduce(
                        out=xr[:],
                        in_=g[:, s0 : s0 + ln, 0:32].rearrange("p s e -> p e s"),
                        axis=mybir.AxisListType.X,
                        op=ADD,
                    )
                    if x_sb is None:
                        x_sb = xr
                    else:
                        nc.vector.tensor_tensor(
                            out=x_sb[:], in0=x_sb[:], in1=xr[:], op=ADD
                        )
                xt_p = xtppool.tile([32, 128], f32, tag="xtp")
                nc.tensor.transpose(xt_p[:], x_sb[:], id_t[:])
                nc.vector.tensor_scalar(
                    out=h1all[:, 128 * t : 128 * (t + 1)],
                    in0=xt_p[:],
                    scalar1=b1_t[:],
                    scalar2=0.0,
                    op0=ADD,
                    op1=MAX,
                )
                if t % 4 == 3:
                    mlp_tail(t // 4)

            # Gather calls; consume each t-block (one block late, software
            # pipelined) once the call containing its last slot is issued.
            next_t = 0
            for ci, (col, cw, n_idx, ns, gs0) in enumerate(CALLS):
                n_slots = (n_idx + 127) // 128
                g = gpool.tile([128, S_MAX, PROJ_W], bf16, tag="g")
                g_tiles[ci] = g
                nc.gpsimd.dma_gather(
                    g[:, :n_slots, :],
                    emb[BIAS:, :],
                    idx_t[:, col : col + cw],
                    n_idx,
                    nreg[n_idx],
                    PROJ_W,
                    single_packet=False,
                    queue_num=ci % 4,
                )
                if stage == "gather":
                    nc.sync.dma_start(
                        out[:, (ci % 16) * 128 : (ci % 16) * 128 + 128],
                        g[0:1, 0:2, :].bitcast(f32).rearrange("a b c -> a (b c)"),
                    )
                    continue
                # blocks fully covered by calls 0..ci, minus one for swp
                done_t = (gs0 + ns) // F
                while next_t < done_t - 1:
                    consume_block(next_t)
                    next_t += 1
            if stage == "full":
                while next_t < NT:
                    consume_block(next_t)
                    next_t += 1
                nc.sync.dma_start(out[:], out_t[:])
    nc.compile()
    return nc


def _get_nc():
    global _nc_cache
    if _nc_cache is None:
        _nc_cache = _build()
    return _nc_cache


def _prep_indices(shard: np.ndarray) -> np.ndarray:
    """[F, B_CORE] int -> [128, IDX_COLS] int16 device layout.

    Global slot s = t*32 + f holds feature f for the 128 batches of t-block
    t; the feature-sum is a DVE reduction over slots. Indices are biased by
    -BIAS, padded with PAD zeros per call (row BIAS stays non-negative so
    the Q7 truncation loop is a no-op), laid out [16, CW] wrapped,
    replicated across the 8 Q7 core groups, and 64 B-aligned per call.
    """
    arr = shard.reshape(F, NT, 128)  # [f, t, b_in]
    stream = arr.transpose(1, 0, 2).astype(np.int64) - BIAS  # [t, f, b_in]
    stream = stream.reshape(N_SLOTS, 128)  # global slot-major
    outa = np.zeros((128, IDX_COLS), np.int16)
    for col, cw, n_idx, ns, gs0 in CALLS:
        lst = np.zeros(n_idx, np.int64)
        lst[: ns * 128] = stream[gs0 : gs0 + ns].reshape(-1)
        lay = lst.reshape(cw, 16).T  # [16, cw]
        outa[:, col : col + cw] = np.tile(lay, (8, 1))
    return outa


def _in_maps(inputs):
    import ml_dtypes

    indices = np.asarray(inputs["indices"])
    emb = np.asarray(inputs["emb"], dtype=np.float32)
    w1 = np.asarray(inputs["w1"], dtype=np.float32)
    b1 = np.asarray(inputs["b1"], dtype=np.float32)
    w2 = np.asarray(inputs["w2"], dtype=np.float32)
    b2 = np.asarray(inputs["b2"], dtype=np.float32)
    w3 = np.asarray(inputs["w3"], dtype=np.float32)
    b3 = np.asarray(inputs["b3"], dtype=np.float32)

    # Fold the first (linear) MLP layer into the table: 256 B bf16 rows.
    projp = np.zeros((INPUT_DIM, PROJ_W), ml_dtypes.bfloat16)
    projp[:, :32] = (emb @ w1.T).astype(ml_dtypes.bfloat16)

    common = {
        "emb": projp,
        "ident": np.eye(128, dtype=np.float32),
        "b1": b1.reshape(32, 1),
        "w2l": np.ascontiguousarray(w2.T).astype(ml_dtypes.bfloat16),
        "b2": b2.reshape(32, 1),
        "w3l": np.ascontiguousarray(w3.T).astype(ml_dtypes.bfloat16),
        "b3": b3.reshape(1, 1),
    }
    in_maps = []
    for c in range(N_CORES):
        shard = indices[:, c * B_CORE : (c + 1) * B_CORE]
        in_maps.append({**common, "idx": _prep_indices(shard)})
    return in_maps


def kernel(**inputs) -> np.ndarray:
    from concourse.bass_utils import run_bass_kernel_spmd

    nc = _get_nc()
    res = run_bass_kernel_spmd(nc, _in_maps(inputs), core_ids=list(range(N_CORES)))
    ys = [np.asarray(res.results[c]["out"]).reshape(B_CORE) for c in range(N_CORES)]
    return np.concatenate(ys).reshape(BATCH, 1).astype(np.float32)


# revision 26
# speedup vs baseline: 1.2517x; 1.0827x over previous
"""NNUE-style embedding-lookup + tiny-MLP kernel for Trainium2 (8 NeuronCores).

Data-parallel over the batch dim: each of the 8 cores handles 2048 of the
16384 batch positions.

Key optimizations:
- The first MLP layer is linear, so the 1 KB embedding rows are folded with
  w1 on the host: proj = emb @ w1.T [49152, 32] in bf16, padded to 128 bf16
  columns (256 B rows -- the dma_gather minimum element size). The device
  gathers 256 B rows instead of 1 KB and the w1 matmul disappears.
- SWDGE descriptor generation (~7.6 ns/index/queue-pair, 4 queues, ~1.4 us
  fixed per call) is the throughput wall; payload bytes are nearly free.
  Descriptors trigger at the END of each call's generation and calls
  dispatch in in-order windows of 4, so the schedule uses a small solo
  leader, a clean 4-call ramp window, full-size middles, and a small last
  window.
- Feature accumulation runs on DVE (strided tensor_reduce over the gathered
  slots + adds), not on PE identity matmuls: the consumer chain per t-block
  is ~1.5 us (2-3 DVE reduces, 1 PE transpose, 1 fused DVE bias+relu into a
  persistent h1 buffer), so the last t-blocks' chains don't pile up after
  the final gather call.
- The 32->32->1 MLP runs in bf16, batched over 512-batch chunks, emitted
  mid-run as soon as each chunk's 4 t-blocks are done.
"""

import numpy as np

INPUT_DIM = 49152
PROJ_W = 128         # padded bf16 proj row: 32 real + 96 pad = 256 B
BATCH = 16384
F = 32               # active features per position
N_CORES = 8
B_CORE = BATCH // N_CORES          # 2048 batch positions per core
BIAS = 16384                       # index bias for int16 gather
PAD = 16                           # per-call idx pad (keeps last index >= 0)
NT = B_CORE // 128                 # 16 t-blocks (128 batches each)
N_SLOTS = NT * F                   # 512 gather slots of 128 lookups

# Call sizes in slots (1 slot = 128 lookups). Calls dispatch in in-order
# windows of 4 (one per SWDGE queue); descriptor generation is superlinear
# in call size (4.6 ns/idx @1040, 5.7 @2064, 7.4 @5648) and large calls
# destabilize the dispatch pipeline, so 16-slot calls are the sweet spot.
# Small solo leader (the first call dispatches alone), 8-slot ramp window,
# small last window so the tail data lands early.
SLOT_SIZES = [4] + [8] * 4 + [16] * 28 + [8] * 3 + [4]
assert sum(SLOT_SIZES) == N_SLOTS


def _call_table():
    """Flat call list: (col, cw, n_idx, n_slots_real, global_slot0)."""
    calls = []
    col = 0
    gs = 0
    for ns in SLOT_SIZES:
        n_idx = ns * 128 + PAD
        cw = n_idx // 16
        stride = ((cw + 31) // 32) * 32  # 64 B-aligned call starts
        calls.append((col, cw, n_idx, ns, gs))
        col += stride
        gs += ns
    return calls, col


CALLS, IDX_COLS = _call_table()


def _ranges():
    """Per t-block: list of (call_idx, local_slot0, length) covering its
    32 slots, split only at call boundaries."""
    out = [[] for _ in range(NT)]
    for ci, (col, cw, n_idx, ns, gs0) in enumerate(CALLS):
        s = 0
        while s < ns:
            t = (gs0 + s) // F
            ln = min(ns - s, F - ((gs0 + s) % F))
            out[t].append((ci, s, ln))
            s += ln
    return out


BLOCK_RANGES = _ranges()

_nc_cache = None


def _build():
    import os
    import concourse.bacc as bacc
    import concourse.mybir as mybir
    import concourse.tile as tile

    stage = os.environ.get("KERNEL_STAGE", "full")

    f32 = mybir.dt.float32
    bf16 = mybir.dt.bfloat16
    i16 = mybir.dt.int16
    AF = mybir.ActivationFunctionType
    ADD = mybir.AluOpType.add
    MAX = mybir.AluOpType.max

    nc = bacc.Bacc(None, target_bir_lowering=False, debug=False, num_swdge_queues=4)
    emb = nc.dram_tensor("emb", [INPUT_DIM, PROJ_W], bf16, kind="ExternalInput")
    idx = nc.dram_tensor("idx", [128, IDX_COLS], i16, kind="ExternalInput")
    ident = nc.dram_tensor("ident", [128, 128], f32, kind="ExternalInput")
    b1 = nc.dram_tensor("b1", [32, 1], f32, kind="ExternalInput")
    w2l = nc.dram_tensor("w2l", [32, 32], bf16, kind="ExternalInput")
    b2 = nc.dram_tensor("b2", [32, 1], f32, kind="ExternalInput")
    w3l = nc.dram_tensor("w3l", [32, 1], bf16, kind="ExternalInput")
    b3 = nc.dram_tensor("b3", [1, 1], f32, kind="ExternalInput")
    out = nc.dram_tensor("out", [1, B_CORE], f32, kind="ExternalOutput")

    S_MAX = max(SLOT_SIZES) + 1  # slots in the largest gather tile (+pad)

    with tile.TileContext(nc) as tc:
        with (
            tc.tile_pool(name="const", bufs=1) as cpool,
            tc.tile_pool(name="g", bufs=12) as gpool,
            tc.tile_pool(name="xs", bufs=3) as xspool,
            tc.tile_pool(name="hs", bufs=4) as hspool,
            tc.tile_pool(name="xtp", bufs=2, space="PSUM") as xtppool,
            tc.tile_pool(name="mp", bufs=4, space="PSUM") as mppool,
        ):
            # idx slices load first; the leader call only needs the first
            # (small) slice, so it lands early (per-range dep tracking).
            idx_t = cpool.tile([128, IDX_COLS], i16)
            first = ((CALLS[0][1] + 31) // 32) * 32
            bounds = [0, first]
            rest = IDX_COLS - first
            for k in range(7):
                bounds.append(first + ((rest * (k + 1) // 7 + 31) // 32) * 32)
            bounds[-1] = IDX_COLS
            for lo, hi in zip(bounds[:-1], bounds[1:]):
                if lo < hi:
                    nc.sync.dma_start(idx_t[:, lo:hi], idx[:, lo:hi])
            id_t = cpool.tile([128, 128], f32)
            nc.sync.dma_start(id_t[:], ident[:])
            b1_t = cpool.tile([32, 1], f32)
            nc.sync.dma_start(b1_t[:], b1[:])
            w2l_t = cpool.tile([32, 32], bf16)
            nc.sync.dma_start(w2l_t[:], w2l[:])
            b2_t = cpool.tile([32, 1], f32)
            nc.sync.dma_start(b2_t[:], b2[:])
            w3l_t = cpool.tile([32, 1], bf16)
            nc.sync.dma_start(w3l_t[:], w3l[:])
            b3_t = cpool.tile([1, 1], f32)
            nc.sync.dma_start(b3_t[:], b3[:])
            out_t = cpool.tile([1, B_CORE], f32)

            # Persistent relu(x@w1.T+b1) buffer (bf16): per-t-block chains
            # end here; the 32->32->1 MLP runs batched per 4 t-blocks.
            h1all = cpool.tile([32, B_CORE], bf16)

            # Pre-load the Tanh activation table (~1.3 us) so it doesn't
            # land in the critical tail at the first real Tanh.
            warm_t = hspool.tile([1, 1], f32, tag="hs")
            nc.scalar.activation(warm_t[:], b3_t[:], AF.Tanh)

            # Shared num_idxs registers: one MOVE each instead of one per
            # call (each MOVE costs ~400 ns of GpSimd sequencer time).
            nreg = {}
            for sz in sorted({c[2] for c in CALLS}):
                nreg[sz] = nc.gpsimd.to_reg(sz)

            g_tiles = [None] * len(CALLS)

            def mlp_tail(k):
                """Batched bf16 32->32->1 MLP over 512 batches (4 t-blocks).

                bf16 matmuls run 1 cycle/row; DVE does the fused bias+relu;
                only the Tanh needs ScalarE.
                """
                sl = slice(512 * k, 512 * (k + 1))
                h2p = mppool.tile([32, 512], f32, tag="mp")
                nc.tensor.matmul(
                    h2p[:], lhsT=w2l_t[:], rhs=h1all[:, sl], start=True, stop=True
                )
                h2s = hspool.tile([32, 512], bf16, tag="h2s")
                nc.vector.tensor_scalar(
                    out=h2s[:], in0=h2p[:], scalar1=b2_t[:], scalar2=0.0,
                    op0=ADD, op1=MAX,
                )
                yp = mppool.tile([1, 512], f32, tag="mp")
                nc.tensor.matmul(
                    yp[:], lhsT=w3l_t[:], rhs=h2s[:], start=True, stop=True
                )
                nc.scalar.activation(out_t[:, sl], yp[:], AF.Tanh, bias=b3_t[:])

            def consume_block(t):
                """Per-t-block consumer: DVE reduces over the gathered slots
                (first 32 columns of each 256 B row), a PE transpose, and a
                fused DVE bias+relu into h1all."""
                x_sb = None
                for ci, s0, ln in BLOCK_RANGES[t]:
                    g = g_tiles[ci]
                    xr = xspool.tile([128, 32], f32, tag="xs")
                    nc.vector.tensor_reduce(
                        out=xr[:],
                        in_=g[:, s0 : s0 + ln, 0:32].rearrange("p s e -> p e s"),
                        axis=mybir.AxisListType.X,
                        op=ADD,
                    )
                    if x_sb is None:
                        x_sb = xr
                    else:
                        nc.vector.tensor_tensor(
                            out=x_sb[:], in0=x_sb[:], in1=xr[:], op=ADD
                        )
                xt_p = xtppool.tile([32, 128], f32, tag="xtp")
                nc.tensor.transpose(xt_p[:], x_sb[:], id_t[:])
                nc.vector.tensor_scalar(
                    out=h1all[:, 128 * t : 128 * (t + 1)],
                    in0=xt_p[:],
                    scalar1=b1_t[:],
                    scalar2=0.0,
                    op0=ADD,
                    op1=MAX,
                )
                if t % 4 == 3:
                    mlp_tail(t // 4)

            # Gather calls; consume each t-block (one block late, software
            # pipelined) once the call containing its last slot is issued.
            next_t = 0
            for ci, (col, cw, n_idx, ns, gs0) in enumerate(CALLS):
                n_slots = (n_idx + 127) // 128
                g = gpool.tile([128, S_MAX, PROJ_W], bf16, tag="g")
                g_tiles[ci] = g
                nc.gpsimd.dma_gather(
                    g[:, :n_slots, :],
                    emb[BIAS:, :],
                    idx_t[:, col : col + cw],
                    n_idx,
                    nreg[n_idx],
                    PROJ_W,
                    single_packet=False,
                    queue_num=(ci + 3) % 4,
                )
                if stage == "gather":
                    nc.sync.dma_start(
                        out[:, (ci % 16) * 128 : (ci % 16) * 128 + 128],
                        g[0:1, 0:2, :].bitcast(f32).rearrange("a b c -> a (b c)"),
                    )
                    continue
                # blocks fully covered by calls 0..ci, minus one for swp
                done_t = (gs0 + ns) // F
                while next_t < done_t - 1:
                    consume_block(next_t)
                    next_t += 1
            if stage == "full":
                while next_t < NT:
                    consume_block(next_t)
                    next_t += 1
                nc.sync.dma_start(out[:], out_t[:])
    nc.compile()
    return nc


def _get_nc():
    global _nc_cache
    if _nc_cache is None:
        _nc_cache = _build()
    return _nc_cache


def _prep_indices(shard: np.ndarray) -> np.ndarray:
    """[F, B_CORE] int -> [128, IDX_COLS] int16 device layout.

    Global slot s = t*32 + f holds feature f for the 128 batches of t-block
    t; the feature-sum is a DVE reduction over slots. Indices are biased by
    -BIAS, padded with PAD zeros per call (row BIAS stays non-negative so
    the Q7 truncation loop is a no-op), laid out [16, CW] wrapped,
    replicated across the 8 Q7 core groups, and 64 B-aligned per call.
    """
    arr = shard.reshape(F, NT, 128)  # [f, t, b_in]
    stream = arr.transpose(1, 0, 2).astype(np.int64) - BIAS  # [t, f, b_in]
    stream = stream.reshape(N_SLOTS, 128)  # global slot-major
    outa = np.zeros((128, IDX_COLS), np.int16)
    for col, cw, n_idx, ns, gs0 in CALLS:
        lst = np.zeros(n_idx, np.int64)
        lst[: ns * 128] = stream[gs0 : gs0 + ns].reshape(-1)
        lay = lst.reshape(cw, 16).T  # [16, cw]
        outa[:, col : col + cw] = np.tile(lay, (8, 1))
    return outa


def _in_maps(inputs):
    import ml_dtypes

    indices = np.asarray(inputs["indices"])
    emb = np.asarray(inputs["emb"], dtype=np.float32)
    w1 = np.asarray(inputs["w1"], dtype=np.float32)
    b1 = np.asarray(inputs["b1"], dtype=np.float32)
    w2 = np.asarray(inputs["w2"], dtype=np.float32)
    b2 = np.asarray(inputs["b2"], dtype=np.float32)
    w3 = np.asarray(inputs["w3"], dtype=np.float32)
    b3 = np.asarray(inputs["b3"], dtype=np.float32)

    # Fold the first (linear) MLP layer into the table: 256 B bf16 rows.
    projp = np.zeros((INPUT_DIM, PROJ_W), ml_dtypes.bfloat16)
    projp[:, :32] = (emb @ w1.T).astype(ml_dtypes.bfloat16)

    common = {
        "emb": projp,
        "ident": np.eye(128, dtype=np.float32),
        "b1": b1.reshape(32, 1),
        "w2l": np.ascontiguousarray(w2.T).astype(ml_dtypes.bfloat16),
        "b2": b2.reshape(32, 1),
        "w3l": np.ascontiguousarray(w3.T).astype(ml_dtypes.bfloat16),
        "b3": b3.reshape(1, 1),
    }
    in_maps = []
    for c in range(N_CORES):
        shard = indices[:, c * B_CORE : (c + 1) * B_CORE]
        in_maps.append({**common, "idx": _prep_indices(shard)})
    return in_maps


def kernel(**inputs) -> np.ndarray:
    from concourse.bass_utils import run_bass_kernel_spmd

    nc = _get_nc()
    res = run_bass_kernel_spmd(nc, _in_maps(inputs), core_ids=list(range(N_CORES)))
    ys = [np.asarray(res.results[c]["out"]).reshape(B_CORE) for c in range(N_CORES)]
    return np.concatenate(ys).reshape(BATCH, 1).astype(np.float32)


# revision 28
# speedup vs baseline: 1.2568x; 1.0041x over previous
"""NNUE-style embedding-lookup + tiny-MLP kernel for Trainium2 (8 NeuronCores).

Data-parallel over the batch dim: each of the 8 cores handles 2048 of the
16384 batch positions.

Key optimizations:
- The first MLP layer is linear, so the 1 KB embedding rows are folded with
  w1 on the host: proj = emb @ w1.T [49152, 32] in bf16, padded to 128 bf16
  columns (256 B rows -- the dma_gather minimum element size). The device
  gathers 256 B rows instead of 1 KB and the w1 matmul disappears.
- SWDGE descriptor generation (~7.6 ns/index/queue-pair, 4 queues, ~1.4 us
  fixed per call) is the throughput wall; payload bytes are nearly free.
  Descriptors trigger at the END of each call's generation and calls
  dispatch in in-order windows of 4, so the schedule uses a small solo
  leader, a clean 4-call ramp window, full-size middles, and a small last
  window.
- Feature accumulation runs on DVE (strided tensor_reduce over the gathered
  slots + adds), not on PE identity matmuls: the consumer chain per t-block
  is ~1.5 us (2-3 DVE reduces, 1 PE transpose, 1 fused DVE bias+relu into a
  persistent h1 buffer), so the last t-blocks' chains don't pile up after
  the final gather call.
- The 32->32->1 MLP runs in bf16, batched over 512-batch chunks, emitted
  mid-run as soon as each chunk's 4 t-blocks are done.
"""

import numpy as np

INPUT_DIM = 49152
PROJ_W = 128         # padded bf16 proj row: 32 real + 96 pad = 256 B
BATCH = 16384
F = 32               # active features per position
N_CORES = 8
B_CORE = BATCH // N_CORES          # 2048 batch positions per core
BIAS = 16384                       # index bias for int16 gather
PAD = 16                           # per-call idx pad (keeps last index >= 0)
NT = B_CORE // 128                 # 16 t-blocks (128 batches each)
N_SLOTS = NT * F                   # 512 gather slots of 128 lookups

# Call sizes in slots (1 slot = 128 lookups). Calls dispatch in in-order
# windows of 4 (one per SWDGE queue); descriptor generation is superlinear
# in call size (4.6 ns/idx @1040, 5.7 @2064, 7.4 @5648) and large calls
# destabilize the dispatch pipeline, so 16-slot calls are the sweet spot.
# Small solo leader (the first call dispatches alone), 8-slot ramp window,
# small last window so the tail data lands early.
SLOT_SIZES = [4] + [8] * 4 + [16] * 28 + [8] * 3 + [4]
assert sum(SLOT_SIZES) == N_SLOTS


def _call_table():
    """Flat call list: (col, cw, n_idx, n_slots_real, global_slot0)."""
    calls = []
    col = 0
    gs = 0
    for ns in SLOT_SIZES:
        n_idx = ns * 128 + PAD
        cw = n_idx // 16
        stride = ((cw + 31) // 32) * 32  # 64 B-aligned call starts
        calls.append((col, cw, n_idx, ns, gs))
        col += stride
        gs += ns
    return calls, col


CALLS, IDX_COLS = _call_table()


def _ranges():
    """Per t-block: list of (call_idx, local_slot0, length) covering its
    32 slots, split only at call boundaries."""
    out = [[] for _ in range(NT)]
    for ci, (col, cw, n_idx, ns, gs0) in enumerate(CALLS):
        s = 0
        while s < ns:
            t = (gs0 + s) // F
            ln = min(ns - s, F - ((gs0 + s) % F))
            out[t].append((ci, s, ln))
            s += ln
    return out


BLOCK_RANGES = _ranges()

_nc_cache = None


def _build():
    import os
    import concourse.bacc as bacc
    import concourse.mybir as mybir
    import concourse.tile as tile

    stage = os.environ.get("KERNEL_STAGE", "full")

    f32 = mybir.dt.float32
    bf16 = mybir.dt.bfloat16
    i16 = mybir.dt.int16
    AF = mybir.ActivationFunctionType
    ADD = mybir.AluOpType.add
    MAX = mybir.AluOpType.max

    # 64 KB/partition descriptor carveout: the default 16 KB gives the SWDGE
    # rings ~128 descriptors per DMA rail, and a 2064-index gather call needs
    # 130/rail -- descriptor generation stalls on ring space mid-call.
    nc = bacc.Bacc(
        None,
        target_bir_lowering=False,
        debug=False,
        num_swdge_queues=4,
        dynamic_dma_scratch_size=65536,
    )
    emb = nc.dram_tensor("emb", [INPUT_DIM, PROJ_W], bf16, kind="ExternalInput")
    idx = nc.dram_tensor("idx", [128, IDX_COLS], i16, kind="ExternalInput")
    ident = nc.dram_tensor("ident", [128, 128], f32, kind="ExternalInput")
    b1 = nc.dram_tensor("b1", [32, 1], f32, kind="ExternalInput")
    w2l = nc.dram_tensor("w2l", [32, 32], bf16, kind="ExternalInput")
    b2 = nc.dram_tensor("b2", [32, 1], f32, kind="ExternalInput")
    w3l = nc.dram_tensor("w3l", [32, 1], bf16, kind="ExternalInput")
    b3 = nc.dram_tensor("b3", [1, 1], f32, kind="ExternalInput")
    out = nc.dram_tensor("out", [1, B_CORE], f32, kind="ExternalOutput")

    S_MAX = max(SLOT_SIZES) + 1  # slots in the largest gather tile (+pad)

    with tile.TileContext(nc) as tc:
        with (
            tc.tile_pool(name="const", bufs=1) as cpool,
            tc.tile_pool(name="g", bufs=12) as gpool,
            tc.tile_pool(name="xs", bufs=3) as xspool,
            tc.tile_pool(name="hs", bufs=4) as hspool,
            tc.tile_pool(name="xtp", bufs=2, space="PSUM") as xtppool,
            tc.tile_pool(name="mp", bufs=4, space="PSUM") as mppool,
        ):
            # idx slices load first; the leader call only needs the first
            # (small) slice, so it lands early (per-range dep tracking).
            idx_t = cpool.tile([128, IDX_COLS], i16)
            first = ((CALLS[0][1] + 31) // 32) * 32
            bounds = [0, first]
            rest = IDX_COLS - first
            for k in range(7):
                bounds.append(first + ((rest * (k + 1) // 7 + 31) // 32) * 32)
            bounds[-1] = IDX_COLS
            for lo, hi in zip(bounds[:-1], bounds[1:]):
                if lo < hi:
                    nc.sync.dma_start(idx_t[:, lo:hi], idx[:, lo:hi])
            id_t = cpool.tile([128, 128], f32)
            nc.sync.dma_start(id_t[:], ident[:])
            b1_t = cpool.tile([32, 1], f32)
            nc.sync.dma_start(b1_t[:], b1[:])
            w2l_t = cpool.tile([32, 32], bf16)
            nc.sync.dma_start(w2l_t[:], w2l[:])
            b2_t = cpool.tile([32, 1], f32)
            nc.sync.dma_start(b2_t[:], b2[:])
            w3l_t = cpool.tile([32, 1], bf16)
            nc.sync.dma_start(w3l_t[:], w3l[:])
            b3_t = cpool.tile([1, 1], f32)
            nc.sync.dma_start(b3_t[:], b3[:])
            out_t = cpool.tile([1, B_CORE], f32)

            # Persistent relu(x@w1.T+b1) buffer (bf16): per-t-block chains
            # end here; the 32->32->1 MLP runs batched per 4 t-blocks.
            h1all = cpool.tile([32, B_CORE], bf16)

            # Pre-load the Tanh activation table (~1.3 us) so it doesn't
            # land in the critical tail at the first real Tanh.
            warm_t = hspool.tile([1, 1], f32, tag="hs")
            nc.scalar.activation(warm_t[:], b3_t[:], AF.Tanh)

            # Shared num_idxs registers: one MOVE each instead of one per
            # call (each MOVE costs ~400 ns of GpSimd sequencer time).
            nreg = {}
            for sz in sorted({c[2] for c in CALLS}):
                nreg[sz] = nc.gpsimd.to_reg(sz)

            g_tiles = [None] * len(CALLS)

            def mlp_tail(k):
                """Batched bf16 32->32->1 MLP over 512 batches (4 t-blocks).

                bf16 matmuls run 1 cycle/row; DVE does the fused bias+relu;
                only the Tanh needs ScalarE.
                """
                sl = slice(512 * k, 512 * (k + 1))
                h2p = mppool.tile([32, 512], f32, tag="mp")
                nc.tensor.matmul(
                    h2p[:], lhsT=w2l_t[:], rhs=h1all[:, sl], start=True, stop=True
                )
                h2s = hspool.tile([32, 512], bf16, tag="h2s")
                nc.vector.tensor_scalar(
                    out=h2s[:], in0=h2p[:], scalar1=b2_t[:], scalar2=0.0,
                    op0=ADD, op1=MAX,
                )
                yp = mppool.tile([1, 512], f32, tag="mp")
                nc.tensor.matmul(
                    yp[:], lhsT=w3l_t[:], rhs=h2s[:], start=True, stop=True
                )
                nc.scalar.activation(out_t[:, sl], yp[:], AF.Tanh, bias=b3_t[:])

            def consume_block(t):
                """Per-t-block consumer: DVE reduces over the gathered slots
                (first 32 columns of each 256 B row), a PE transpose, and a
                fused DVE bias+relu into h1all."""
                x_sb = None
                for ci, s0, ln in BLOCK_RANGES[t]:
                    g = g_tiles[ci]
                    xr = xspool.tile([128, 32], f32, tag="xs")
                    nc.vector.tensor_reduce(
                        out=xr[:],
                        in_=g[:, s0 : s0 + ln, 0:32].rearrange("p s e -> p e s"),
                        axis=mybir.AxisListType.X,
                        op=ADD,
                    )
                    if x_sb is None:
                        x_sb = xr
                    else:
                        nc.vector.tensor_tensor(
                            out=x_sb[:], in0=x_sb[:], in1=xr[:], op=ADD
                        )
                xt_p = xtppool.tile([32, 128], f32, tag="xtp")
                nc.tensor.transpose(xt_p[:], x_sb[:], id_t[:])
                nc.vector.tensor_scalar(
                    out=h1all[:, 128 * t : 128 * (t + 1)],
                    in0=xt_p[:],
                    scalar1=b1_t[:],
                    scalar2=0.0,
                    op0=ADD,
                    op1=MAX,
                )
                if t % 4 == 3:
                    mlp_tail(t // 4)

            # Gather calls; consume each t-block (one block late, software
            # pipelined) once the call containing its last slot is issued.
            next_t = 0
            for ci, (col, cw, n_idx, ns, gs0) in enumerate(CALLS):
                n_slots = (n_idx + 127) // 128
                g = gpool.tile([128, S_MAX, PROJ_W], bf16, tag="g")
                g_tiles[ci] = g
                nc.gpsimd.dma_gather(
                    g[:, :n_slots, :],
                    emb[BIAS:, :],
                    idx_t[:, col : col + cw],
                    n_idx,
                    nreg[n_idx],
                    PROJ_W,
                    single_packet=False,
                    queue_num=ci % 4,
                )
                if stage == "gather":
                    nc.sync.dma_start(
                        out[:, (ci % 16) * 128 : (ci % 16) * 128 + 128],
                        g[0:1, 0:2, :].bitcast(f32).rearrange("a b c -> a (b c)"),
                    )
                    continue
                # blocks fully covered by calls 0..ci, minus one for swp
                done_t = (gs0 + ns) // F
                while next_t < done_t - 1:
                    consume_block(next_t)
                    next_t += 1
            if stage == "full":
                while next_t < NT:
                    consume_block(next_t)
                    next_t += 1
                nc.sync.dma_start(out[:], out_t[:])
    nc.compile()
    return nc


def _get_nc():
    global _nc_cache
    if _nc_cache is None:
        _nc_cache = _build()
    return _nc_cache


def _prep_indices(shard: np.ndarray) -> np.ndarray:
    """[F, B_CORE] int -> [128, IDX_COLS] int16 device layout.

    Global slot s = t*32 + f holds feature f for the 128 batches of t-block
    t; the feature-sum is a DVE reduction over slots. Indices are biased by
    -BIAS, padded with PAD zeros per call (row BIAS stays non-negative so
    the Q7 truncation loop is a no-op), laid out [16, CW] wrapped,
    replicated across the 8 Q7 core groups, and 64 B-aligned per call.
    """
    arr = shard.reshape(F, NT, 128)  # [f, t, b_in]
    stream = arr.transpose(1, 0, 2).astype(np.int64) - BIAS  # [t, f, b_in]
    stream = stream.reshape(N_SLOTS, 128)  # global slot-major
    outa = np.zeros((128, IDX_COLS), np.int16)
    for col, cw, n_idx, ns, gs0 in CALLS:
        lst = np.zeros(n_idx, np.int64)
        lst[: ns * 128] = stream[gs0 : gs0 + ns].reshape(-1)
        lay = lst.reshape(cw, 16).T  # [16, cw]
        outa[:, col : col + cw] = np.tile(lay, (8, 1))
    return outa


def _in_maps(inputs):
    import ml_dtypes

    indices = np.asarray(inputs["indices"])
    emb = np.asarray(inputs["emb"], dtype=np.float32)
    w1 = np.asarray(inputs["w1"], dtype=np.float32)
    b1 = np.asarray(inputs["b1"], dtype=np.float32)
    w2 = np.asarray(inputs["w2"], dtype=np.float32)
    b2 = np.asarray(inputs["b2"], dtype=np.float32)
    w3 = np.asarray(inputs["w3"], dtype=np.float32)
    b3 = np.asarray(inputs["b3"], dtype=np.float32)

    # Fold the first (linear) MLP layer into the table: 256 B bf16 rows.
    projp = np.zeros((INPUT_DIM, PROJ_W), ml_dtypes.bfloat16)
    projp[:, :32] = (emb @ w1.T).astype(ml_dtypes.bfloat16)

    common = {
        "emb": projp,
        "ident": np.eye(128, dtype=np.float32),
        "b1": b1.reshape(32, 1),
        "w2l": np.ascontiguousarray(w2.T).astype(ml_dtypes.bfloat16),
        "b2": b2.reshape(32, 1),
        "w3l": np.ascontiguousarray(w3.T).astype(ml_dtypes.bfloat16),
        "b3": b3.reshape(1, 1),
    }
    in_maps = []
    for c in range(N_CORES):
        shard = indices[:, c * B_CORE : (c + 1) * B_CORE]
        in_maps.append({**common, "idx": _prep_indices(shard)})
    return in_maps


def kernel(**inputs) -> np.ndarray:
    from concourse.bass_utils import run_bass_kernel_spmd

    nc = _get_nc()
    res = run_bass_kernel_spmd(nc, _in_maps(inputs), core_ids=list(range(N_CORES)))
    ys = [np.asarray(res.results[c]["out"]).reshape(B_CORE) for c in range(N_CORES)]
    return np.concatenate(ys).reshape(BATCH, 1).astype(np.float32)
